# revision 18
# baseline (speedup 1.0000x reference)
"""Trainium2 Bass kernel for nn_DipolePredictorSE3 (SE(3)-invariant sparse
graph attention + pooled MLP head).

Contract: kernel(**inputs) takes FULL unsharded inputs (B=16 graphs) and
returns the FULL [16, 3] float32 output. Internally shards 2 graphs per
NeuronCore across 8 cores (data parallel over batch), runs one SPMD Bass
program via run_bass_kernel_spmd, and finishes the tiny pooled MLP head on
host.

Math notes (vs reference):
  - neigh = adj | (adj@adj > 0) | eye  ==  ((adj|I) @ (adj|I)) > 0  exactly.
    M = adj|I is 0/1, exact in fp8; P = M@M accumulated in f32 PSUM is an
    exact integer count, so the mask min(P,1) is exact. P uses fp8
    DoubleRow matmuls (2 fp8 weights per PE cell -> 2 MACs/cycle).
  - content scores fold Wq,Wk: S[j,i] = f_i^T G f_j with G = Wq Wk^T/sqrt(D),
    computed as kt^T qt with kt = G f^T (device matmul, bf16 hi/lo split)
    and qt = f^T (fp16 input).
  - b_r2 (and any constant shift) cancels in softmax -> dropped.
  - With b_r1 == 0 (the setup default), rbias(d) = c*d with
    c = sum_{r: w1r>0} w1r*w2r, since d = dist > 0. (A b_r1 != 0 input
    falls back to an exact numpy path.)
  - rb = |c|*d = sqrt(c^2*d2 + eps) in ONE ACT op (eps absorbs the small
    negative excursions of the bf16-split d2, replacing clamp+ln+exp).
    All sqrt ops are batched before all exp ops so the ACT table set is
    loaded exactly twice (sqrt set, then exp set), not per-tile.
  - Softmax is computed unnormalized with a fixed logit shift (exact in the
    ratio); the denominator comes from a ones-column in the padded V.
    Scores are produced transposed (S[j,i]) which makes the aggregation
    matmul transpose-free; valid because mask and rbias are symmetric.
  - All four (graph, col-half) aggregations accumulate into ONE PSUM bank
    via [128, 24] zero-padded V tiles whose 6-column slot selects the
    output rows; the single [24, 512] result DMAs straight to HBM.
"""

import os
import sys

import numpy as np

for _p in ("/opt/trn_rl_repo", "/root/.axon_site/_ro/trn_rl_repo"):
    if os.path.isdir(_p) and _p not in sys.path:
        sys.path.insert(0, _p)

import concourse.bass as bass  # noqa: E402
import concourse.mybir as mybir  # noqa: E402
from concourse.bass_utils import run_bass_kernel_spmd  # noqa: E402
from concourse.tile import TileContext  # noqa: E402

B, N, D = 16, 1024, 5
NCORES = 8
GPC = B // NCORES  # graphs per core
NT = GPC * 16  # [128, 512] score tiles per core
AF = mybir.ActivationFunctionType
OP = mybir.AluOpType
PM = mybir.MatmulPerfMode

SHIFT = 12.0  # constant logit shift folded into exp bias (exact in softmax)
EPS_REL = 2e-2  # d2 negative-excursion guard (bf16-split d2 can dip ~-4e-3)

last_results = None  # stashed BassKernelResults for test.py introspection


def _bf16_split(x):
    """Return (hi, lo) bf16 split of float32 array x (x ~= hi + lo)."""
    bf16 = mybir.dt.np(mybir.dt.bfloat16)
    hi = x.astype(bf16)
    lo = (x - hi.astype(np.float32)).astype(bf16)
    return hi, lo


def _split_matmul_waits(nc):
    """Walrus's fused-matmul ISA struct holds only one sync wait; hoist
    extra waits onto preceding same-engine NoOps (identical sync
    semantics: engine queues execute in order)."""
    nid = [0]
    for blk in nc.m.functions[0].blocks:
        new_insts = []
        for ins in blk.instructions:
            si = ins.sync_info
            tn = type(ins).__name__
            splittable = tn not in (
                "InstNoOp", "InstAllEngineBarrier",
                "InstEventSemaphore", "InstTriggerDma",
                "InstLoadActFuncSet",
            ) and getattr(ins, "engine", None) in (
                mybir.EngineType.PE, mybir.EngineType.Activation,
                mybir.EngineType.DVE, mybir.EngineType.Pool,
                mybir.EngineType.SP,
            )
            if (
                splittable
                and si is not None
                and si.on_wait
                and len(si.on_wait) > 1
            ):
                waits = list(si.on_wait)
                for w in waits[:-1]:
                    nop = mybir.InstNoOp(
                        name=f"{ins.name}-wsplit{nid[0]}",
                        engine=ins.engine,
                        bass_nofuse=True,
                    )
                    nid[0] += 1
                    nop.sync_info = mybir.SyncInfo(on_wait=[w], on_update=[])
                    new_insts.append(nop)
                ins.sync_info = mybir.SyncInfo(
                    on_wait=[waits[-1]], on_update=list(si.on_update)
                )
            new_insts.append(ins)
        blk.instructions = new_insts


def _build(c_val):
    """Build the SPMD Bass program (per core: GPC graphs)."""
    nc = bass.Bass()
    f32 = mybir.dt.float32
    bf16 = mybir.dt.bfloat16
    fp8 = mybir.dt.float8e4
    fp16 = mybir.dt.float16

    c2 = float(c_val) * float(c_val)
    eps = EPS_REL * max(c2, 1e-12)

    m8l = nc.dram_tensor("m8l", [GPC, 128, 8, N], fp8, kind="ExternalInput")
    extl = nc.dram_tensor("extl", [GPC, 20, N], bf16, kind="ExternalInput")
    extr = nc.dram_tensor("extr", [GPC, 20, N], bf16, kind="ExternalInput")
    qt16 = nc.dram_tensor("qt16", [GPC, 5, N], fp16, kind="ExternalInput")
    fs = nc.dram_tensor("fs", [GPC, 15, N], bf16, kind="ExternalInput")
    gs = nc.dram_tensor("gs", [15, 5], bf16, kind="ExternalInput")
    # per (graph, col-half) zero-padded [Wv; ones] slot matrices
    ws = nc.dram_tensor("ws", [GPC, 2, 15, 24], bf16, kind="ExternalInput")
    sdiag = nc.dram_tensor("sdiag", [128, 128], fp16, kind="ExternalInput")
    u_out = nc.dram_tensor("u_out", [24, 512], f32, kind="ExternalOutput")

    with TileContext(nc) as tc:
        with (
            tc.tile_pool(name="pconst", bufs=1) as pconst,
            tc.tile_pool(name="psmall", bufs=2) as psmall,
            tc.tile_pool(name="pmt", bufs=2) as pmt,
            tc.tile_pool(name="pel", bufs=6) as pel,
            tc.tile_pool(name="ppd", bufs=2, space="PSUM") as ppd,
            tc.tile_pool(name="ppp", bufs=2, space="PSUM") as ppp,
            tc.tile_pool(name="ppc", bufs=4, space="PSUM") as ppc,
        ):
            # ---- constants / small loads ----
            sdiag_t = pconst.tile([128, 128], fp16, name="sdiag_t")
            nc.sync.dma_start(sdiag_t, sdiag[:, :])
            gs_t = pconst.tile([15, 5], bf16, name="gs_t")
            nc.sync.dma_start(gs_t, gs[:, :])

            def bias_tile(val, nm):
                t = pconst.tile([128, 1], f32, name=nm)
                nc.vector.memset(t, float(val))
                return t

            b_eps = bias_tile(eps, "b_eps")
            b_shift = bias_tile(-SHIFT, "b_shift")
            # b_exp is (re)written only after the LAST sqrt (see below) so
            # every exp depends on every sqrt: the scheduler then cannot
            # interleave the two ACT table sets (2 table loads, not 44).
            b_exp = pconst.tile([128, 1], f32, name="b_exp")

            warm = pconst.tile([128, 512], bf16, name="warm")
            nc.vector.memset(warm, 0.0)

            # ---- bulk loads: small tensors first, mask chunks last ----
            mts, extls, extrs, qts, fss, wss = [], [], [], [], [], []
            for g in range(GPC):
                q = nc.sync if g == 0 else nc.scalar
                el = psmall.tile([128, N], bf16, tag="extl", name=f"extl{g}")
                nc.gpsimd.memset(el, 0.0)
                q.dma_start(el[0:20, :], extl[g])
                extls.append(el)
                er = psmall.tile([128, N], bf16, tag="extr", name=f"extr{g}")
                nc.gpsimd.memset(er, 0.0)
                q.dma_start(er[0:20, :], extr[g])
                extrs.append(er)
                qt = psmall.tile([128, N], fp16, tag="qt", name=f"qt{g}")
                nc.gpsimd.memset(qt, 0.0)
                q.dma_start(qt[0:5, :], qt16[g])
                qts.append(qt)
                ft = psmall.tile([15, N], bf16, tag="fs", name=f"fs{g}")
                q.dma_start(ft, fs[g])
                fss.append(ft)
                wpair = []
                for ih in range(2):
                    wt = psmall.tile([15, 24], bf16, tag="ws",
                                     name=f"ws{g}_{ih}")
                    q.dma_start(wt, ws[g, ih])
                    wpair.append(wt)
                wss.append(wpair)
            for g in range(GPC):
                mt = pmt.tile([128, 8, N], fp8, tag="mt", name=f"mt{g}")
                qeng = nc.sync if g == 0 else nc.scalar
                for s2 in range(2):
                    qeng.dma_start(mt[:, 4 * s2 : 4 * s2 + 4, :],
                                   m8l[g, :, 4 * s2 : 4 * s2 + 4, :])
                mts.append(mt)

            # probe: measure dma_start_transpose cost on scratch tiles
            tpsrc = pconst.tile([128, 128], fp16, name="tpsrc")
            nc.vector.memset(tpsrc, 1.0)
            tpdst = pconst.tile([128, 8, 128], fp16, name="tpdst")
            for i in range(8):
                nc.sync.dma_start_transpose(tpdst[:, i, :], tpsrc)

            # ---- PE warmup while DMAs stream (keeps HAM at 8/8) ----
            for i in range(20):
                wp = ppc.tile([128, 512], f32, tag="pc", name=f"warmp{i}")
                nc.tensor.matmul(wp, warm[:, 0:128], warm, start=True,
                                 stop=True)

            # ---- prep: kt = G f^T (bf16 hi/lo), per-(g,ih) padded V ----
            kts = []
            for g in range(GPC):
                kt = psmall.tile([128, N], fp16, tag="kt", name=f"kt{g}")
                nc.gpsimd.memset(kt, 0.0)
                for ih in range(2):
                    sl = slice(ih * 512, (ih + 1) * 512)
                    pk = ppc.tile([5, 512], f32, tag="pc", name=f"pk{g}_{ih}")
                    nc.tensor.matmul(pk, gs_t, fss[g][:, sl], start=True,
                                     stop=True)
                    nc.vector.tensor_copy(kt[0:5, sl], pk)
                kts.append(kt)

            vexts = {}
            for g in range(GPC):
                for ih in range(2):
                    k = 2 * g + ih
                    pv = ppc.tile([128, 8, 24], f32, tag="pc",
                                  name=f"pv{g}_{ih}")
                    for jc in range(8):
                        nc.tensor.matmul(
                            pv[:, jc, :],
                            fss[g][:, jc * 128 : (jc + 1) * 128],
                            wss[g][ih],
                            start=True, stop=True,
                        )
                    vx = psmall.tile([128, 8, 24], fp16, tag="vext",
                                     name=f"vext{g}_{ih}")
                    nc.vector.tensor_copy(vx, pv)
                    nc.vector.memset(vx[:, :, 6 * k + 5 : 6 * k + 6], 1.0)
                    vexts[(g, ih)] = vx

            # ---- phase A: dist + mask per tile; sqrt on ACT, min on DVE ----
            tiles = [(g, jc, ih) for g in range(GPC) for jc in range(8)
                     for ih in range(2)]
            rbs, m01s = {}, {}
            for t, (g, jc, ih) in enumerate(tiles):
                jsl = slice(jc * 128, (jc + 1) * 128)
                isl = slice(ih * 512, (ih + 1) * 512)
                pd2 = ppd.tile([128, 512], f32, tag="pd", name=f"pd_{t}")
                nc.tensor.matmul(pd2, extls[g][:, jsl], extrs[g][:, isl],
                                 start=True, stop=True)
                pp = ppp.tile([128, 512], f32, tag="pp", name=f"pp_{t}")
                mt = mts[g]
                for s2 in range(4):
                    nc.tensor.matmul(
                        pp,
                        mt[:, 2 * s2 : 2 * s2 + 2, jsl],
                        mt[:, 2 * s2 : 2 * s2 + 2, isl],
                        start=(s2 == 0), stop=(s2 == 3),
                        perf_mode=PM.DoubleRow,
                    )
                # rb = |c| * dist = sqrt(c2*d2 + eps)
                rb = pel.tile([128, 512], fp16, tag="rb", bufs=NT,
                              name=f"rb_{t}")
                nc.scalar.activation(rb, pd2, AF.Sqrt,
                                     bias=b_eps[:128, :], scale=c2)
                rbs[t] = rb
                m01 = pel.tile([128, 512], fp16, tag="m01", bufs=NT,
                               name=f"m01_{t}")
                nc.vector.tensor_single_scalar(m01, pp, 1.0, OP.min)
                m01s[t] = m01

            # phase gate: b_exp = rb_last[:,0:1]*0 + (-SHIFT)
            nc.vector.scalar_tensor_tensor(
                b_exp, rbs[NT - 1][:, 0:1], 0.0, b_shift,
                OP.mult, OP.add,
            )

            # keepalive: full-K dummy matmuls bridge the exp table load
            for i in range(4):
                ka = ppd.tile([128, 512], f32, tag="pd", name=f"keep{i}")
                nc.tensor.matmul(ka, warm[:, 0:128], warm, start=True,
                                 stop=True)

            # ---- phase B: content + rbias, exp, mask-mul, aggregate ----
            aggp_full = ppd.tile([128, 512], f32, tag="pd", name="aggp")
            aggp = aggp_full[0:24, :]
            n_agg = [0]

            def emit_agg(t):
                g, jc, ih = tiles[t]
                nc.tensor.matmul(
                    aggp,
                    vexts[(g, ih)][:, jc, :],
                    ems.pop(t),
                    start=(n_agg[0] == 0), stop=(n_agg[0] == NT - 1),
                    skip_group_check=True,
                )
                n_agg[0] += 1

            ems = {}
            for t, (g, jc, ih) in enumerate(tiles):
                jsl = slice(jc * 128, (jc + 1) * 128)
                isl = slice(ih * 512, (ih + 1) * 512)
                pc = ppc.tile([128, 512], f32, tag="pc", name=f"pc_{t}")
                nc.tensor.matmul(pc, kts[g][:, jsl], qts[g][:, isl],
                                 start=True, stop=False)
                nc.tensor.matmul(pc, sdiag_t, rbs.pop(t),
                                 start=False, stop=True)
                et = pel.tile([128, 512], fp16, tag="et", bufs=4,
                              name=f"et_{t}")
                nc.scalar.activation(et, pc, AF.Exp,
                                     bias=b_exp[:128, :])
                em = pel.tile([128, 512], fp16, tag="em", bufs=6,
                              name=f"em_{t}")
                nc.vector.tensor_mul(em, et, m01s.pop(t))
                ems[t] = em
                if t >= 4:
                    emit_agg(t - 4)
            for t in range(NT - 4, NT):
                emit_agg(t)

            u_sb = pel.tile([24, 512], f32, tag="usb", name="u_sb")
            nc.vector.tensor_copy(u_sb, aggp)
            nc.sync.dma_start(u_out[:, :], u_sb)
    _split_matmul_waits(nc)
    return nc


def _host_reference(feats, coors, adj_bool, Wq, Wk, Wv, Wo,
                    w_r1, b_r1, w_r2, b_r2, w1, b1, w2, b2):
    """Exact numpy fallback (general radial MLP path)."""
    f64 = np.float64
    feats64 = feats.astype(f64)
    a = adj_bool.astype(f64)
    adj2 = np.einsum("bij,bjk->bik", a, a) > 0
    eye = np.eye(N, dtype=bool)[None]
    neigh = adj_bool | adj2 | eye
    q = feats64 @ Wq.astype(f64)
    k = feats64 @ Wk.astype(f64)
    v = feats64 @ Wv.astype(f64)
    rel = coors[:, :, None, :].astype(f64) - coors[:, None, :, :].astype(f64)
    dist = np.sqrt((rel * rel).sum(-1) + 1e-8)
    h = np.maximum(dist[..., None] * w_r1[0] + b_r1, 0.0)
    rbias = (h @ w_r2.astype(f64))[..., 0] + b_r2[0]
    scores = np.einsum("bid,bjd->bij", q, k) / np.sqrt(D) + rbias
    scores = np.where(neigh, scores, -1e9)
    scores -= scores.max(axis=-1, keepdims=True)
    e = np.exp(scores)
    attn = e / e.sum(axis=-1, keepdims=True)
    agg = np.einsum("bij,bjd->bid", attn, v)
    x = feats64 + agg @ Wo.astype(f64)
    pooled = x.mean(axis=1)
    hdn = np.maximum(pooled @ w1.astype(f64) + b1, 0.0)
    return (hdn @ w2.astype(f64) + b2).astype(np.float32)


def kernel(
    feats, coors, adj_mat, Wq, Wk, Wv, Wo,
    w_r1, b_r1, w_r2, b_r2, w1, b1, w2, b2,
):
    global last_results
    f32 = np.float32
    fp8np = mybir.dt.np(mybir.dt.float8e4)
    fp16np = mybir.dt.np(mybir.dt.float16)
    bf16np = mybir.dt.np(mybir.dt.bfloat16)

    feats = np.asarray(feats, dtype=f32)
    coors = np.asarray(coors, dtype=f32)
    adj = np.asarray(adj_mat).astype(bool)
    Wq = np.asarray(Wq, f32); Wk = np.asarray(Wk, f32)
    Wv = np.asarray(Wv, f32); Wo = np.asarray(Wo, f32)
    w_r1 = np.asarray(w_r1, f32); b_r1 = np.asarray(b_r1, f32)
    w_r2 = np.asarray(w_r2, f32); b_r2 = np.asarray(b_r2, f32)
    w1 = np.asarray(w1, f32); b1 = np.asarray(b1, f32)
    w2 = np.asarray(w2, f32); b2 = np.asarray(b2, f32)

    # radial MLP fast-path constant: rbias(d) = c*d (+const) when b_r1 == 0
    fast_path = bool(np.all(b_r1 == 0.0))
    if not fast_path:
        return _host_reference(feats, coors, adj, Wq, Wk, Wv, Wo,
                               w_r1, b_r1, w_r2, b_r2, w1, b1, w2, b2)
    w1v = w_r1[0]
    w2v = w_r2[:, 0]
    c_val = float(np.sum(np.where(w1v > 0, w1v * w2v, 0.0)))

    # ---- host layout prep (no model compute beyond O(B*N)) ----
    eye = np.eye(N, dtype=bool)
    m8 = (adj | eye[None]).astype(fp8np)  # [B,N,N] fp8 {0,1}
    # DMA-friendly: partition p holds rows {s*128+p}, contiguous 8KB lines
    m8l = np.ascontiguousarray(
        m8.reshape(B, 8, 128, N).transpose(0, 2, 1, 3)
    )  # [B,128,8,N]

    fT = np.ascontiguousarray(feats.transpose(0, 2, 1))  # [B,5,N]
    fhi, flo = _bf16_split(fT)
    fs = np.concatenate([fhi, fhi, flo], axis=1)  # [B,15,N]
    qt16 = fT.astype(fp16np)

    n2 = (coors * coors).sum(-1)  # [B,N]
    ones = np.ones_like(n2)
    ct = coors.transpose(0, 2, 1)  # [B,3,N]
    extL = np.concatenate([-2.0 * ct, n2[:, None], ones[:, None]], axis=1)
    extR = np.concatenate([ct, ones[:, None], n2[:, None]], axis=1)  # [B,5,N]
    lhi, llo = _bf16_split(extL)
    rhi, rlo = _bf16_split(extR)
    # sum over 20 rows = LhiRhi + LhiRlo + LloRhi + LloRlo
    extl20 = np.concatenate([lhi, lhi, llo, llo], axis=1)  # [B,20,N]
    extr20 = np.concatenate([rhi, rlo, rhi, rlo], axis=1)

    G = (Wq @ Wk.T / np.sqrt(D)).astype(f32)  # [5,5]
    Ghi, Glo = _bf16_split(G)
    # kt[d,j] = sum_e G[d,e] f[j,e]; lhsT rows pair with fs rows [fhi,fhi,flo]
    gsm = np.concatenate([Ghi.T, Glo.T, Ghi.T], axis=0)  # [15,5]

    Whi, Wlo = _bf16_split(Wv)
    wsb = np.concatenate([Whi, Wlo, Whi], axis=0)  # [15,5] pairs with fs
    # ws[g, ih]: zero-padded so slot k=2g+ih receives [v; (ones col later)]
    ws = np.zeros((B, 2, 15, 24), dtype=np.float32)
    for b in range(B):
        g01 = b % GPC
        for ih in range(2):
            k = 2 * g01 + ih
            ws[b, ih, :, 6 * k : 6 * k + 5] = wsb

    sgn = np.float32(1.0 if c_val >= 0 else -1.0)
    sdiag = (sgn * np.eye(128, dtype=np.float32)).astype(fp16np)

    nc = _build(c_val)

    in_maps = []
    for core in range(NCORES):
        gs_idx = [core * GPC + g for g in range(GPC)]
        in_maps.append(
            {
                "m8l": np.ascontiguousarray(m8l[gs_idx]),
                "extl": np.ascontiguousarray(extl20[gs_idx]).astype(bf16np),
                "extr": np.ascontiguousarray(extr20[gs_idx]).astype(bf16np),
                "qt16": np.ascontiguousarray(qt16[gs_idx]),
                "fs": np.ascontiguousarray(fs[gs_idx]).astype(bf16np),
                "gs": gsm.astype(bf16np),
                "ws": np.ascontiguousarray(ws[gs_idx]).astype(bf16np),
                "sdiag": sdiag,
            }
        )

    trace = bool(os.environ.get("BASS_TRACE"))
    res = run_bass_kernel_spmd(nc, in_maps, list(range(NCORES)), trace=trace)
    last_results = res

    # ---- host finish: normalize, pool, tiny MLP head ----
    # u_out [24,512]: slot k=2*g+ih -> rows 6k..6k+5 = [agg d=0..4; denom],
    # cols = node range ih*512..ih*512+511 of per-core graph g.
    u = np.zeros((B, D + 1, N), dtype=np.float64)
    for core in range(NCORES):
        uo = res.results[core]["u_out"].astype(np.float64)
        for g01 in range(GPC):
            bidx = core * GPC + g01
            for ih in range(2):
                k = 2 * g01 + ih
                u[bidx, :, ih * 512 : (ih + 1) * 512] = uo[6 * k : 6 * k + 6]
    aggT = u[:, 0:D, :] / u[:, D : D + 1, :]  # [B,5,N]
    agg_mean = aggT.mean(axis=2)  # [B,5]
    pooled = feats.mean(axis=1) + agg_mean @ Wo  # [B,5]
    hdn = np.maximum(pooled @ w1 + b1, 0.0)
    out = hdn @ w2 + b2
    return out.astype(f32)
